# revision 61
# baseline (speedup 1.0000x reference)
"""Trainium2 Bass kernel for the CIN block:
out[b,o,k] = sum_{h,m} W[o, h*M+m] * xl[b,h,k] * x0[b,m,k] + bias[o]

Strategy: data-parallel over batch across 8 cores (32 batches/core,
8 groups of 4).  Per group the feature map fmap[c,(gi,kk)] is built by
DVE tensor_mul and consumed by ONE uninterrupted PE GEMM stream
(lhsT = W^T chunk [128c,128o], rhs = fmap chunk [128c,512], fp32 PSUM,
2 o-chunks, bias added during PSUM evacuation by ScalarE).

v2 channel layout: chunk q = (hg in 4) * 8 + (mg in 8); partition
j = hi*8 + mi with h = 16*hg + hi, m = 8*mg + mi.  The DVE operands
then factor per group into just TWO [128, 4096] SBUF tiles:
  xl2: 4 hg-blocks of 1024 cols, xl rows broadcast over mi and
       duplicated over the chunk-parity (so one tensor_mul spans a
       2-chunk burst);
  x0r: 4 mg-pair-blocks of 1024 cols, x0 rows broadcast over hi.
Burst b=(hg,mp): fmap[:, b*1024:+1024] = xl2[hg-block] * x0r[mp-block].
This cuts input DMA from ~4.5 MB/group (full xl broadcast) to 2 MB.

Trailing-chunk fp8 path (NFP8=8 chunks = hg 3; rel err 1.81e-2 vs the
2e-2 gate, measured exactly by tools/err_sim.py): DVE multiplies
(xl/8 * x0) in bf16 (the x0 operand read element-interleaved through a
rearranged AP), ScalarE casts to e4m3, and the PE runs DoubleRow
matmuls on pair-interleaved operands with W*8 weights.  The 8 groups
are processed as two QUADS: all four bf16 sections, then one DR block
in which each 256-column DoubleRow weight load (not FWL-accelerated,
so otherwise exposed) serves four N=512 matmuls; all 8 PSUM banks hold
the quad's accumulators (warmup aliases into the psg ring).  Ordering
rules that matter: per quad, PSUM evacuations are emitted BEFORE the
next quad's builds so the ScalarE FIFO never parks casts ahead of
evacs (PSUM recycling), and the final quad's oc1 evacuations split
DVE/ScalarE with output DMA on the idle sync queue (short tail).
"""

import sys
import types
import warnings

warnings.filterwarnings("ignore")

import numpy as np
import ml_dtypes

B, M, H, K, O = 256, 64, 64, 128, 256
C = H * M                  # 4096 channels
NCORES = 8
BPC = B // NCORES          # 32 batches per core
GRP = 4                    # batches per group (moving dim = GRP*K = 512)
NG = BPC // GRP            # 8 groups per core
KB = GRP * K               # 512
NCHUNK = C // 128          # 32 contraction chunks
NFP8 = 8                   # trailing chunks done in fp8 e4m3 DoubleRow
FP8_S = 8.0                # W*S and fmap/S keep both operands in e4m3 normals
NBF = NCHUNK - NFP8        # bf16 chunks
WPP = 4                    # wt chunks per DMA piece
NWP = (NBF + WPP - 1) // WPP
WARM = 54                  # PE warmup matmuls (HAM un-throttle)

_BF16 = ml_dtypes.bfloat16
_E4M3 = ml_dtypes.float8_e4m3fn

LAST_EXEC_NS = None


def _install_ntff_hook():
    try:
        from antenv.axon_hooks import get_axon_ntff_profile_hook  # noqa: F401
        return
    except ImportError:
        pass
    try:
        from trn_agent_boot.trn_boot import _ntff_profile_via_ctypes
        hook = _ntff_profile_via_ctypes('/opt/axon/libaxon_pjrt.so')
    except Exception:
        hook = None
    m = types.ModuleType('antenv.axon_hooks')
    m.get_axon_ntff_profile_hook = lambda: hook
    m.set_axon_ntff_profile_hook = lambda h: None
    sys.modules['antenv.axon_hooks'] = m


_NC_CACHE = {}


def _build_program():
    if "nc" in _NC_CACHE:
        return _NC_CACHE["nc"]
    import concourse.bacc as bacc
    import concourse.tile as tile
    import concourse.mybir as mybir

    dt = mybir.dt
    nc = bacc.Bacc("TRN2", target_bir_lowering=False, debug=False)

    xl2_d = nc.dram_tensor("xl2", [NG, 4, 128, 1024], dt.bfloat16,
                           kind="ExternalInput").ap()
    x0r_d = nc.dram_tensor("x0r", [NG, 4, 128, 1024], dt.bfloat16,
                           kind="ExternalInput").ap()
    wt_d = nc.dram_tensor("wt", [128, NBF * O], dt.bfloat16,
                          kind="ExternalInput").ap()
    wt8_d = nc.dram_tensor("wt8", [128, NFP8 * O], dt.float8e4,
                           kind="ExternalInput").ap() if NFP8 else None
    bias_d = nc.dram_tensor("bias_t", [128, 2], dt.float32,
                            kind="ExternalInput").ap()
    # outputs leave the chip in bf16 (error contribution ~1e-3, negligible
    # vs the fp8 budget); the host upconverts to fp32.
    out_d = nc.dram_tensor("out", [BPC, O, K], dt.bfloat16,
                           kind="ExternalOutput").ap()

    with tile.TileContext(nc) as tc:
        with tc.tile_pool(name="const", bufs=1) as cpool, \
             tc.tile_pool(name="grp", bufs=3) as gpool, \
             tc.tile_pool(name="fmapp", bufs=4) as fpool, \
             tc.tile_pool(name="fm8p", bufs=6) as fm8pool, \
             tc.tile_pool(name="f8bp", bufs=3) as f8bpool, \
             tc.tile_pool(name="outp", bufs=3) as opool, \
             tc.tile_pool(name="psg", bufs=4, space="PSUM") as psg:

            wu = cpool.tile([128, 128], dt.bfloat16)
            nc.vector.memset(wu[:], 0.0)

            # PE warmup: pulls the HAM clock-gate to 8/8 and covers the
            # initial input-DMA latency.  The warmup accumulator aliases into
            # the psg0 ring (all 8 PSUM banks go to the quad's accumulators).
            ps_w = psg.tile([128, KB], dt.float32, name="psx_warm", tag="psg0")
            for wi in range(WARM):
                nc.tensor.matmul(ps_w[:, 0:128], wu[:, :], wu[:, :],
                                 start=(wi == 0), stop=(wi == WARM - 1))

            xl2_t = [None] * NG
            x0r_t = [None] * NG
            wt_t = [None] * NWP
            wt8 = None
            bias_t = None

            def alloc_group(g):
                xl2_t[g] = gpool.tile([128, 4096], dt.bfloat16,
                                      name=f"xl2_{g}", tag="xl2")
                x0r_t[g] = gpool.tile([128, 4096], dt.bfloat16,
                                      name=f"x0r_{g}", tag="x0r")

            def dma_piece(g, which, q, lo=0, hi=1024, eng=None):
                t, d = ((xl2_t, xl2_d) if which == 'xl2' else (x0r_t, x0r_d))
                (eng or nc.sync).dma_start(t[g][:, 1024 * q + lo:1024 * q + hi],
                                           d[g, q][:, lo:hi])

            def dma_wt_piece(q, lo=0, hi=None, eng=None):
                w = min(WPP, NBF - WPP * q) * O
                if hi is None:
                    hi = w
                if wt_t[q] is None:
                    wt_t[q] = cpool.tile([128, w], dt.bfloat16,
                                         name=f"wt_{q}", tag=f"wt{q}")
                base = q * WPP * O
                (eng or nc.sync).dma_start(wt_t[q][:, lo:hi],
                                           wt_d[:, base + lo:base + hi])

            def dma_group(g, split=False):
                alloc_group(g)
                eng = nc.scalar if split else None
                for q in range(4):
                    dma_piece(g, 'x0r', q)
                for q in range(4):
                    dma_piece(g, 'xl2', q, eng=eng)

            # startup DMA order: tiny pieces feeding the first matmuls
            # first (descriptor generation is the early bottleneck), then
            # few BIG transfers to keep all 16 SDMA queues saturated.
            # x0r goes on the sync HWDGE queue, xl2 + weights on the
            # (otherwise idle early) scalar HWDGE queue.
            alloc_group(0)
            dma_piece(0, 'x0r', 0, 0, 512)
            dma_piece(0, 'xl2', 0, 0, 512, eng=nc.scalar)
            dma_wt_piece(0, 0, 256, eng=nc.scalar)
            dma_piece(0, 'x0r', 0, 512, 1024)
            dma_piece(0, 'xl2', 0, 512, 1024, eng=nc.scalar)
            dma_wt_piece(0, 256, None, eng=nc.scalar)
            nc_bias = cpool.tile([128, 2], dt.float32)
            bias_t = nc_bias
            nc.sync.dma_start(bias_t[:], bias_d[:])
            dma_piece(0, 'x0r', 1)
            dma_wt_piece(1, eng=nc.scalar)
            dma_piece(0, 'x0r', 2)
            dma_piece(0, 'xl2', 1, eng=nc.scalar)
            dma_piece(0, 'x0r', 3)
            dma_wt_piece(2, eng=nc.scalar)
            dma_piece(0, 'xl2', 2, eng=nc.scalar)
            dma_wt_piece(3, eng=nc.scalar)
            dma_piece(0, 'xl2', 3, eng=nc.scalar)
            for q in range(4, NWP):
                dma_wt_piece(q, eng=nc.scalar)
            if NFP8:
                wt8 = cpool.tile([128, NFP8 * O], dt.float8e4)
                nc.scalar.dma_start(wt8[:], wt8_d[:])

            fmap_t = [None] * NG
            fmap8_t = [None] * NG
            pso_t = [None] * NG

            def emit_fmap_build(g, split_first=False):
                fmap_t[g] = fpool.tile([128, NBF * KB], dt.bfloat16,
                                       name=f"fmap_{g}", tag="fmap")
                if NFP8:
                    fmap8_t[g] = fm8pool.tile([128, NFP8 * KB], dt.float8e4,
                                              name=f"fmap8_{g}", tag="fmap8")
                xl2, x0r = xl2_t[g], x0r_t[g]
                for b in range(NBF // 2):        # 2-chunk tensor_mul bursts
                    hg, mp = divmod(b, 4)
                    sl_x = slice(1024 * hg, 1024 * (hg + 1))
                    sl_o = slice(1024 * mp, 1024 * (mp + 1))
                    dst = fmap_t[g][:, 1024 * b:1024 * (b + 1)]
                    if b == 0 and split_first:
                        nc.vector.tensor_mul(dst[:, :512], xl2[:, :512],
                                             x0r[:, :512])
                        nc.vector.tensor_mul(dst[:, 512:], xl2[:, 512:1024],
                                             x0r[:, 512:1024])
                    else:
                        nc.vector.tensor_mul(dst, xl2[:, sl_x], x0r[:, sl_o])
                for j in range(NFP8 // 2):       # fp8 bursts (pair-interleaved)
                    # DVE multiply fully contiguous in bf16 (plane order);
                    # the ScalarE cast to fp8 does the element-interleave by
                    # reading f8b through a rearranged AP.
                    f8b = f8bpool.tile([128, 1024], dt.bfloat16,
                                       name=f"f8b_{g}_{j}", tag="f8b")
                    nc.vector.tensor_mul(f8b[:], xl2[:, 3 * 1024:4 * 1024],
                                         x0r[:, 1024 * j:1024 * (j + 1)])
                    dst = fmap8_t[g][:, 1024 * j:1024 * (j + 1)]
                    nc.scalar.activation(dst,
                                         f8b[:, :].rearrange(
                                             "p (two n) -> p n two", two=2),
                                         mybir.ActivationFunctionType.Identity)

            def emit_evac(g, oc, dma_sync=False, on_dve=False):
                osb = opool.tile([128, KB], dt.bfloat16,
                                 name=f"osb_{g}_{oc}", tag=f"osb{oc}")
                if on_dve:
                    nc.vector.tensor_scalar_add(osb[:], pso_t[g][oc][:],
                                                bias_t[:, oc:oc + 1])
                else:
                    nc.scalar.activation(osb[:], pso_t[g][oc][:],
                                         mybir.ActivationFunctionType.Identity,
                                         bias=bias_t[:, oc:oc + 1])
                dst = out_d[GRP * g:GRP * (g + 1), 128 * oc:128 * (oc + 1), :] \
                    .rearrange("g o k -> o g k")
                eng = nc.sync if dma_sync else nc.scalar
                eng.dma_start(dst, osb[:, :].rearrange("o (g k) -> o g k", k=K))

            def alloc_pso(g):
                pso_t[g] = [psg.tile([128, KB], dt.float32,
                                     name=f"psg_{g}_{oc}", tag=f"psg{oc}")
                            for oc in range(2)]

            def bf_mm(g, p, oc, stop):
                wtile = wt_t[p // WPP]
                wof = (p % WPP) * O
                nc.tensor.matmul(pso_t[g][oc][:],
                                 wtile[:, wof + 128 * oc:wof + 128 * (oc + 1)],
                                 fmap_t[g][:, KB * p:KB * (p + 1)],
                                 start=(p == 0), stop=stop)

            def dr_mm(g, j, oc, stop):
                # chunk pair element-interleaved in fmap8 (fp8 pair is
                # 16-bit adjacent for the DoubleRow moving stream)
                rhs = fmap8_t[g][:, 1024 * j:1024 * (j + 1)] \
                    .rearrange("p (n two) -> p two n", two=2)
                lhsT = wt8[:, 512 * j + 256 * oc:512 * j + 256 * (oc + 1)] \
                    .rearrange("p (two o) -> p two o", two=2)
                nc.tensor.matmul(pso_t[g][oc][:], lhsT, rhs,
                                 start=False, stop=stop,
                                 perf_mode=mybir.MatmulPerfMode.DoubleRow)

            def emit_gemm(g, oc_split=False):
                alloc_pso(g)
                if oc_split:
                    # last group: finish oc=0 entirely first so its
                    # evacuation + output DMA overlap oc=1's matmuls.
                    for oc in range(2):
                        for p in range(NBF):
                            bf_mm(g, p, oc, stop=(p == NBF - 1))
                        emit_evac(g, oc)
                else:
                    for p in range(NBF):
                        for oc in range(2):
                            bf_mm(g, p, oc, stop=(p == NBF - 1))
                    for oc in range(2):
                        emit_evac(g, oc)

            def emit_gemm_quad(g0, last=False):
                # all four groups' bf16 sections, then one DR block in which
                # each DoubleRow weight load serves four N=512 matmuls.
                gs = [g0, g0 + 1, g0 + 2, g0 + 3]
                for g in gs:
                    alloc_pso(g)
                for g in gs:
                    for p in range(NBF):
                        for oc in range(2):
                            bf_mm(g, p, oc, stop=False)
                for oc in range(2):
                    for j in range(NFP8 // 2):
                        for g in gs:
                            dr_mm(g, j, oc, stop=(j == NFP8 // 2 - 1))
                    # evac right after each oc closes: the last quad's oc0
                    # outputs overlap its oc1 DR matmuls; the final oc1
                    # evacuations split across DVE/ScalarE and their output
                    # DMAs go out on the (idle by then) sync queue.
                    for i, g in enumerate(gs):
                        fin = last and oc == 1
                        emit_evac(g, oc, dma_sync=fin,
                                  on_dve=(fin and i % 2 == 0))

            if NFP8 == 0:
                dma_group(1)
                emit_fmap_build(0, split_first=True)
                for g in range(NG):
                    if g + 2 < NG:
                        dma_group(g + 2)
                    if g + 1 < NG:
                        emit_fmap_build(g + 1)
                    emit_gemm(g, oc_split=(g == NG - 1))
            else:
                dma_group(1, split=True)
                emit_fmap_build(0, split_first=True)
                dma_group(2, split=True)
                emit_fmap_build(1)
                dma_group(3, split=True)
                emit_fmap_build(2)
                emit_fmap_build(3)
                # quad 0; its evacuations precede the second quad's builds'
                # fp8 casts in the ScalarE FIFO (PSUM banks recycle on time).
                emit_gemm_quad(0)
                dma_group(4, split=True)
                dma_group(5, split=True)
                emit_fmap_build(4)
                dma_group(6, split=True)
                emit_fmap_build(5)
                dma_group(7, split=True)
                emit_fmap_build(6)
                emit_fmap_build(7)
                emit_gemm_quad(4, last=True)

    nc.compile()
    _NC_CACHE["nc"] = nc
    return nc


def _host_prep(x0, xl, W, b):
    xlb = xl.astype(_BF16)
    x0b = x0.astype(_BF16)
    xg = xlb.reshape(NCORES, NG, GRP, H, K)
    og = x0b.reshape(NCORES, NG, GRP, M, K)

    # xl2[c,g][j=(hi,mi), hg*1024 + dup*512 + gi*128 + kk] = xl[b, 16hg+hi, kk]
    # (the fp8 hg=3 block is identical in layout, just pre-scaled by 1/S)
    xgs = xg.astype(np.float32).reshape(NCORES, NG, GRP, 4, 16, K)
    if NFP8:
        xgs[:, :, :, 3] /= FP8_S
    t = xgs.astype(_BF16).transpose(0, 1, 4, 3, 2, 5)
    t = np.broadcast_to(t[:, :, :, None, :, None, :, :],
                        (NCORES, NG, 16, 8, 4, 2, GRP, K))
    xl2 = np.ascontiguousarray(t).reshape(NCORES, NG, 128, 4096)

    # x0r[c,g][j, mp*1024 + pair*512 + gi*128 + kk] = x0[b, 8*(2mp+pair)+mi, kk]
    t = og.reshape(NCORES, NG, GRP, 4, 2, 8, K).transpose(0, 1, 5, 3, 4, 2, 6)
    t = np.broadcast_to(t[:, :, None, :, :, :, :, :],
                        (NCORES, NG, 16, 8, 4, 2, GRP, K))
    x0r = np.ascontiguousarray(t).reshape(NCORES, NG, 128, 4096)

    # W[o, c] with c=(16hg+hi)*64 + 8mg+mi -> wt[j=(hi,mi), (hg*8+mg)*O + o]
    Wm = W[:, :, 0]
    Wr = Wm.reshape(O, 4, 16, 8, 8)
    wtf = np.ascontiguousarray(Wr.transpose(2, 4, 1, 3, 0)).reshape(128, 32 * O)
    wt = wtf[:, :NBF * O].astype(_BF16)
    if NFP8:
        w8 = (wtf[:, NBF * O:] * FP8_S).reshape(128, NFP8 // 2, 2, 2, 128)
        wt8 = np.ascontiguousarray(w8.transpose(0, 1, 3, 2, 4)) \
            .reshape(128, NFP8 * O).astype(_E4M3)
    else:
        wt8 = np.zeros((128, 0), dtype=_E4M3)

    bias_t = np.ascontiguousarray(b.reshape(2, 128).T.astype(np.float32))

    # piece views [NC, NG, 4, 128, 1024]
    def piece(a):
        return np.ascontiguousarray(
            a.reshape(NCORES, NG, 128, 4, 1024).transpose(0, 1, 3, 2, 4))

    return piece(xl2), piece(x0r), wt, wt8, bias_t


def kernel(x0, xl, k, W, b, _trace=False):
    global LAST_EXEC_NS
    _install_ntff_hook()
    import concourse.bass_utils as bass_utils

    x0 = np.asarray(x0, dtype=np.float32)
    xl = np.asarray(xl, dtype=np.float32)
    W = np.asarray(W, dtype=np.float32)
    b = np.asarray(b, dtype=np.float32)

    nc = _build_program()
    xl2, x0r, wt, wt8, bias_t = _host_prep(x0, xl, W, b)
    in_maps = [
        {"xl2": xl2[c], "x0r": x0r[c], "wt": wt, "bias_t": bias_t}
        for c in range(NCORES)
    ]
    if NFP8:
        for c in range(NCORES):
            in_maps[c]["wt8"] = wt8
    res = bass_utils.run_bass_kernel_spmd(
        nc, in_maps, core_ids=list(range(NCORES)), trace=_trace)
    LAST_EXEC_NS = res.exec_time_ns

    out = np.concatenate(
        [np.asarray(res.results[c]["out"]).astype(np.float32)[None]
         for c in range(NCORES)], axis=0)
    return np.ascontiguousarray(out.reshape(B, O, K))


# revision 62
# speedup vs baseline: 1.0361x; 1.0361x over previous
"""Trainium2 Bass kernel for the CIN block:
out[b,o,k] = sum_{h,m} W[o, h*M+m] * xl[b,h,k] * x0[b,m,k] + bias[o]

Strategy: data-parallel over batch across 8 cores (32 batches/core,
8 groups of 4).  Per group the feature map fmap[c,(gi,kk)] is built by
DVE tensor_mul and consumed by ONE uninterrupted PE GEMM stream
(lhsT = W^T chunk [128c,128o], rhs = fmap chunk [128c,512], fp32 PSUM,
2 o-chunks, bias added during PSUM evacuation by ScalarE).

v2 channel layout: chunk q = (hg in 4) * 8 + (mg in 8); partition
j = hi*8 + mi with h = 16*hg + hi, m = 8*mg + mi.  The DVE operands
then factor per group into just TWO [128, 4096] SBUF tiles:
  xl2: 4 hg-blocks of 1024 cols, xl rows broadcast over mi and
       duplicated over the chunk-parity (so one tensor_mul spans a
       2-chunk burst);
  x0r: 4 mg-pair-blocks of 1024 cols, x0 rows broadcast over hi.
Burst b=(hg,mp): fmap[:, b*1024:+1024] = xl2[hg-block] * x0r[mp-block].
This cuts input DMA from ~4.5 MB/group (full xl broadcast) to 2 MB.

Trailing-chunk fp8 path (NFP8=8 chunks = hg 3; rel err 1.81e-2 vs the
2e-2 gate, measured exactly by tools/err_sim.py): DVE multiplies
(xl/8 * x0) in bf16 (the x0 operand read element-interleaved through a
rearranged AP), ScalarE casts to e4m3, and the PE runs DoubleRow
matmuls on pair-interleaved operands with W*8 weights.  The 8 groups
are processed as two QUADS: all four bf16 sections, then one DR block
in which each 256-column DoubleRow weight load (not FWL-accelerated,
so otherwise exposed) serves four N=512 matmuls; all 8 PSUM banks hold
the quad's accumulators (warmup aliases into the psg ring).  Ordering
rules that matter: per quad, PSUM evacuations are emitted BEFORE the
next quad's builds so the ScalarE FIFO never parks casts ahead of
evacs (PSUM recycling), and the final quad's oc1 evacuations split
DVE/ScalarE with output DMA on the idle sync queue (short tail).
"""

import sys
import types
import warnings

warnings.filterwarnings("ignore")

import numpy as np
import ml_dtypes

B, M, H, K, O = 256, 64, 64, 128, 256
C = H * M                  # 4096 channels
NCORES = 8
BPC = B // NCORES          # 32 batches per core
GRP = 4                    # batches per group (moving dim = GRP*K = 512)
NG = BPC // GRP            # 8 groups per core
KB = GRP * K               # 512
NCHUNK = C // 128          # 32 contraction chunks
NFP8 = 8                   # trailing chunks done in fp8 e4m3 DoubleRow
FP8_S = 8.0                # W*S and fmap/S keep both operands in e4m3 normals
NBF = NCHUNK - NFP8        # bf16 chunks
WPP = 4                    # wt chunks per DMA piece
NWP = (NBF + WPP - 1) // WPP
WARM = 54                  # PE warmup matmuls (HAM un-throttle)

_BF16 = ml_dtypes.bfloat16
_E4M3 = ml_dtypes.float8_e4m3fn

LAST_EXEC_NS = None


def _install_ntff_hook():
    try:
        from antenv.axon_hooks import get_axon_ntff_profile_hook  # noqa: F401
        return
    except ImportError:
        pass
    try:
        from trn_agent_boot.trn_boot import _ntff_profile_via_ctypes
        hook = _ntff_profile_via_ctypes('/opt/axon/libaxon_pjrt.so')
    except Exception:
        hook = None
    m = types.ModuleType('antenv.axon_hooks')
    m.get_axon_ntff_profile_hook = lambda: hook
    m.set_axon_ntff_profile_hook = lambda h: None
    sys.modules['antenv.axon_hooks'] = m


_NC_CACHE = {}


def _build_program():
    if "nc" in _NC_CACHE:
        return _NC_CACHE["nc"]
    import concourse.bacc as bacc
    import concourse.tile as tile
    import concourse.mybir as mybir

    dt = mybir.dt
    nc = bacc.Bacc("TRN2", target_bir_lowering=False, debug=False)

    xl2_d = nc.dram_tensor("xl2", [NG, 4, 128, 1024], dt.bfloat16,
                           kind="ExternalInput").ap()
    x0r_d = nc.dram_tensor("x0r", [NG, 4, 128, 1024], dt.bfloat16,
                           kind="ExternalInput").ap()
    wt_d = nc.dram_tensor("wt", [128, NBF * O], dt.bfloat16,
                          kind="ExternalInput").ap()
    wt8_d = nc.dram_tensor("wt8", [128, NFP8 * O], dt.float8e4,
                           kind="ExternalInput").ap() if NFP8 else None
    bias_d = nc.dram_tensor("bias_t", [128, 2], dt.float32,
                            kind="ExternalInput").ap()
    # outputs leave the chip in bf16 (error contribution ~1e-3, negligible
    # vs the fp8 budget); the host upconverts to fp32.
    out_d = nc.dram_tensor("out", [BPC, O, K], dt.bfloat16,
                           kind="ExternalOutput").ap()

    with tile.TileContext(nc) as tc:
        with tc.tile_pool(name="const", bufs=1) as cpool, \
             tc.tile_pool(name="grp", bufs=3) as gpool, \
             tc.tile_pool(name="fmapp", bufs=4) as fpool, \
             tc.tile_pool(name="fm8p", bufs=6) as fm8pool, \
             tc.tile_pool(name="f8bp", bufs=3) as f8bpool, \
             tc.tile_pool(name="outp", bufs=3) as opool, \
             tc.tile_pool(name="psg", bufs=4, space="PSUM") as psg:

            wu = cpool.tile([128, 128], dt.bfloat16)
            nc.vector.memset(wu[:], 0.0)

            # PE warmup: pulls the HAM clock-gate to 8/8 and covers the
            # initial input-DMA latency.  The warmup accumulator aliases into
            # the psg0 ring (all 8 PSUM banks go to the quad's accumulators).
            ps_w = psg.tile([128, KB], dt.float32, name="psx_warm", tag="psg0")
            for wi in range(WARM):
                nc.tensor.matmul(ps_w[:, 0:128], wu[:, :], wu[:, :],
                                 start=(wi == 0), stop=(wi == WARM - 1))

            xl2_t = [None] * NG
            x0r_t = [None] * NG
            wt_t = [None] * NWP
            wt8 = None
            bias_t = None

            def alloc_group(g):
                xl2_t[g] = gpool.tile([128, 4096], dt.bfloat16,
                                      name=f"xl2_{g}", tag="xl2")
                x0r_t[g] = gpool.tile([128, 4096], dt.bfloat16,
                                      name=f"x0r_{g}", tag="x0r")

            def dma_piece(g, which, q, lo=0, hi=1024, eng=None):
                t, d = ((xl2_t, xl2_d) if which == 'xl2' else (x0r_t, x0r_d))
                (eng or nc.sync).dma_start(t[g][:, 1024 * q + lo:1024 * q + hi],
                                           d[g, q][:, lo:hi])

            def dma_wt_piece(q, lo=0, hi=None, eng=None):
                w = min(WPP, NBF - WPP * q) * O
                if hi is None:
                    hi = w
                if wt_t[q] is None:
                    wt_t[q] = cpool.tile([128, w], dt.bfloat16,
                                         name=f"wt_{q}", tag=f"wt{q}")
                base = q * WPP * O
                (eng or nc.sync).dma_start(wt_t[q][:, lo:hi],
                                           wt_d[:, base + lo:base + hi])

            def dma_group(g, split=False):
                alloc_group(g)
                eng = nc.scalar if split else None
                for q in range(4):
                    dma_piece(g, 'x0r', q)
                for q in range(4):
                    dma_piece(g, 'xl2', q, eng=eng)

            # startup DMA order: tiny pieces feeding the first matmuls
            # first (descriptor generation is the early bottleneck), then
            # few BIG transfers to keep all 16 SDMA queues saturated.
            # x0r goes on the sync HWDGE queue, xl2 + weights on the
            # (otherwise idle early) scalar HWDGE queue.
            alloc_group(0)
            dma_piece(0, 'x0r', 0, 0, 512)
            dma_piece(0, 'xl2', 0, 0, 512, eng=nc.scalar)
            dma_wt_piece(0, 0, 256, eng=nc.scalar)
            dma_piece(0, 'x0r', 0, 512, 1024)
            dma_piece(0, 'xl2', 0, 512, 1024, eng=nc.scalar)
            dma_wt_piece(0, 256, None, eng=nc.scalar)
            nc_bias = cpool.tile([128, 2], dt.float32)
            bias_t = nc_bias
            nc.sync.dma_start(bias_t[:], bias_d[:])
            dma_piece(0, 'x0r', 1)
            dma_wt_piece(1, eng=nc.scalar)
            dma_piece(0, 'x0r', 2)
            dma_piece(0, 'xl2', 1, eng=nc.scalar)
            dma_piece(0, 'x0r', 3)
            dma_wt_piece(2, eng=nc.scalar)
            dma_piece(0, 'xl2', 2, eng=nc.scalar)
            dma_wt_piece(3, eng=nc.scalar)
            dma_piece(0, 'xl2', 3, eng=nc.scalar)
            for q in range(4, NWP):
                dma_wt_piece(q, eng=nc.scalar)
            if NFP8:
                wt8 = cpool.tile([128, NFP8 * O], dt.float8e4)
                nc.scalar.dma_start(wt8[:], wt8_d[:])

            fmap_t = [None] * NG
            fmap8_t = [None] * NG
            pso_t = [None] * NG

            def emit_fmap_build(g, split_first=False):
                fmap_t[g] = fpool.tile([128, NBF * KB], dt.bfloat16,
                                       name=f"fmap_{g}", tag="fmap")
                if NFP8:
                    fmap8_t[g] = fm8pool.tile([128, NFP8 * KB], dt.float8e4,
                                              name=f"fmap8_{g}", tag="fmap8")
                xl2, x0r = xl2_t[g], x0r_t[g]
                for b in range(NBF // 2):        # 2-chunk tensor_mul bursts
                    hg, mp = divmod(b, 4)
                    sl_x = slice(1024 * hg, 1024 * (hg + 1))
                    sl_o = slice(1024 * mp, 1024 * (mp + 1))
                    dst = fmap_t[g][:, 1024 * b:1024 * (b + 1)]
                    if b == 0 and split_first:
                        nc.vector.tensor_mul(dst[:, :512], xl2[:, :512],
                                             x0r[:, :512])
                        nc.vector.tensor_mul(dst[:, 512:], xl2[:, 512:1024],
                                             x0r[:, 512:1024])
                    else:
                        nc.vector.tensor_mul(dst, xl2[:, sl_x], x0r[:, sl_o])
                for j in range(NFP8 // 2):       # fp8 bursts (pair-interleaved)
                    # DVE multiply fully contiguous in bf16 (plane order);
                    # the ScalarE cast to fp8 does the element-interleave by
                    # reading f8b through a rearranged AP.
                    f8b = f8bpool.tile([128, 1024], dt.bfloat16,
                                       name=f"f8b_{g}_{j}", tag="f8b")
                    nc.vector.tensor_mul(f8b[:], xl2[:, 3 * 1024:4 * 1024],
                                         x0r[:, 1024 * j:1024 * (j + 1)])
                    dst = fmap8_t[g][:, 1024 * j:1024 * (j + 1)]
                    nc.scalar.activation(dst,
                                         f8b[:, :].rearrange(
                                             "p (two n) -> p n two", two=2),
                                         mybir.ActivationFunctionType.Identity)

            def emit_evac(g, oc, dma_sync=False, on_dve=False):
                osb = opool.tile([128, KB], dt.bfloat16,
                                 name=f"osb_{g}_{oc}", tag=f"osb{oc}")
                if on_dve:
                    nc.vector.tensor_scalar_add(osb[:], pso_t[g][oc][:],
                                                bias_t[:, oc:oc + 1])
                else:
                    nc.scalar.activation(osb[:], pso_t[g][oc][:],
                                         mybir.ActivationFunctionType.Identity,
                                         bias=bias_t[:, oc:oc + 1])
                dst = out_d[GRP * g:GRP * (g + 1), 128 * oc:128 * (oc + 1), :] \
                    .rearrange("g o k -> o g k")
                eng = nc.sync if dma_sync else nc.scalar
                eng.dma_start(dst, osb[:, :].rearrange("o (g k) -> o g k", k=K))

            def alloc_pso(g):
                pso_t[g] = [psg.tile([128, KB], dt.float32,
                                     name=f"psg_{g}_{oc}", tag=f"psg{oc}")
                            for oc in range(2)]

            def bf_mm(g, p, oc, stop):
                wtile = wt_t[p // WPP]
                wof = (p % WPP) * O
                nc.tensor.matmul(pso_t[g][oc][:],
                                 wtile[:, wof + 128 * oc:wof + 128 * (oc + 1)],
                                 fmap_t[g][:, KB * p:KB * (p + 1)],
                                 start=(p == 0), stop=stop)

            def dr_mm(g, j, oc, stop):
                # chunk pair element-interleaved in fmap8 (fp8 pair is
                # 16-bit adjacent for the DoubleRow moving stream)
                rhs = fmap8_t[g][:, 1024 * j:1024 * (j + 1)] \
                    .rearrange("p (n two) -> p two n", two=2)
                lhsT = wt8[:, 512 * j + 256 * oc:512 * j + 256 * (oc + 1)] \
                    .rearrange("p (two o) -> p two o", two=2)
                nc.tensor.matmul(pso_t[g][oc][:], lhsT, rhs,
                                 start=False, stop=stop,
                                 perf_mode=mybir.MatmulPerfMode.DoubleRow)

            def emit_gemm(g, oc_split=False):
                alloc_pso(g)
                if oc_split:
                    # last group: finish oc=0 entirely first so its
                    # evacuation + output DMA overlap oc=1's matmuls.
                    for oc in range(2):
                        for p in range(NBF):
                            bf_mm(g, p, oc, stop=(p == NBF - 1))
                        emit_evac(g, oc)
                else:
                    for p in range(NBF):
                        for oc in range(2):
                            bf_mm(g, p, oc, stop=(p == NBF - 1))
                    for oc in range(2):
                        emit_evac(g, oc)

            def emit_gemm_quad(g0, last=False):
                # all four groups' bf16 sections, then one DR block in which
                # each DoubleRow weight load serves four N=512 matmuls.
                gs = [g0, g0 + 1, g0 + 2, g0 + 3]
                for g in gs:
                    alloc_pso(g)
                for g in gs:
                    for p in range(NBF):
                        for oc in range(2):
                            bf_mm(g, p, oc, stop=False)
                for oc in range(2):
                    for j in range(NFP8 // 2):
                        for g in gs:
                            dr_mm(g, j, oc, stop=(j == NFP8 // 2 - 1))
                    # evac right after each oc closes: the last quad's oc0
                    # outputs overlap its oc1 DR matmuls; the final oc1
                    # evacuations split across DVE/ScalarE and their output
                    # DMAs go out on the (idle by then) sync queue.
                    for i, g in enumerate(gs):
                        fin = last and oc == 1
                        emit_evac(g, oc, dma_sync=fin,
                                  on_dve=(fin and i % 2 == 0))

            if NFP8 == 0:
                dma_group(1)
                emit_fmap_build(0, split_first=True)
                for g in range(NG):
                    if g + 2 < NG:
                        dma_group(g + 2)
                    if g + 1 < NG:
                        emit_fmap_build(g + 1)
                    emit_gemm(g, oc_split=(g == NG - 1))
            else:
                dma_group(1, split=True)
                emit_fmap_build(0, split_first=True)
                dma_group(2)
                emit_fmap_build(1)
                dma_group(3)
                emit_fmap_build(2)
                emit_fmap_build(3)
                # quad 0; its evacuations precede the second quad's builds'
                # fp8 casts in the ScalarE FIFO (PSUM banks recycle on time).
                emit_gemm_quad(0)
                dma_group(4)
                dma_group(5)
                emit_fmap_build(4)
                dma_group(6)
                emit_fmap_build(5)
                dma_group(7)
                emit_fmap_build(6)
                emit_fmap_build(7)
                emit_gemm_quad(4, last=True)

    nc.compile()
    _NC_CACHE["nc"] = nc
    return nc


def _host_prep(x0, xl, W, b):
    xlb = xl.astype(_BF16)
    x0b = x0.astype(_BF16)
    xg = xlb.reshape(NCORES, NG, GRP, H, K)
    og = x0b.reshape(NCORES, NG, GRP, M, K)

    # xl2[c,g][j=(hi,mi), hg*1024 + dup*512 + gi*128 + kk] = xl[b, 16hg+hi, kk]
    # (the fp8 hg=3 block is identical in layout, just pre-scaled by 1/S)
    xgs = xg.astype(np.float32).reshape(NCORES, NG, GRP, 4, 16, K)
    if NFP8:
        xgs[:, :, :, 3] /= FP8_S
    t = xgs.astype(_BF16).transpose(0, 1, 4, 3, 2, 5)
    t = np.broadcast_to(t[:, :, :, None, :, None, :, :],
                        (NCORES, NG, 16, 8, 4, 2, GRP, K))
    xl2 = np.ascontiguousarray(t).reshape(NCORES, NG, 128, 4096)

    # x0r[c,g][j, mp*1024 + pair*512 + gi*128 + kk] = x0[b, 8*(2mp+pair)+mi, kk]
    t = og.reshape(NCORES, NG, GRP, 4, 2, 8, K).transpose(0, 1, 5, 3, 4, 2, 6)
    t = np.broadcast_to(t[:, :, None, :, :, :, :, :],
                        (NCORES, NG, 16, 8, 4, 2, GRP, K))
    x0r = np.ascontiguousarray(t).reshape(NCORES, NG, 128, 4096)

    # W[o, c] with c=(16hg+hi)*64 + 8mg+mi -> wt[j=(hi,mi), (hg*8+mg)*O + o]
    Wm = W[:, :, 0]
    Wr = Wm.reshape(O, 4, 16, 8, 8)
    wtf = np.ascontiguousarray(Wr.transpose(2, 4, 1, 3, 0)).reshape(128, 32 * O)
    wt = wtf[:, :NBF * O].astype(_BF16)
    if NFP8:
        w8 = (wtf[:, NBF * O:] * FP8_S).reshape(128, NFP8 // 2, 2, 2, 128)
        wt8 = np.ascontiguousarray(w8.transpose(0, 1, 3, 2, 4)) \
            .reshape(128, NFP8 * O).astype(_E4M3)
    else:
        wt8 = np.zeros((128, 0), dtype=_E4M3)

    bias_t = np.ascontiguousarray(b.reshape(2, 128).T.astype(np.float32))

    # piece views [NC, NG, 4, 128, 1024]
    def piece(a):
        return np.ascontiguousarray(
            a.reshape(NCORES, NG, 128, 4, 1024).transpose(0, 1, 3, 2, 4))

    return piece(xl2), piece(x0r), wt, wt8, bias_t


def kernel(x0, xl, k, W, b, _trace=False):
    global LAST_EXEC_NS
    _install_ntff_hook()
    import concourse.bass_utils as bass_utils

    x0 = np.asarray(x0, dtype=np.float32)
    xl = np.asarray(xl, dtype=np.float32)
    W = np.asarray(W, dtype=np.float32)
    b = np.asarray(b, dtype=np.float32)

    nc = _build_program()
    xl2, x0r, wt, wt8, bias_t = _host_prep(x0, xl, W, b)
    in_maps = [
        {"xl2": xl2[c], "x0r": x0r[c], "wt": wt, "bias_t": bias_t}
        for c in range(NCORES)
    ]
    if NFP8:
        for c in range(NCORES):
            in_maps[c]["wt8"] = wt8
    res = bass_utils.run_bass_kernel_spmd(
        nc, in_maps, core_ids=list(range(NCORES)), trace=_trace)
    LAST_EXEC_NS = res.exec_time_ns

    out = np.concatenate(
        [np.asarray(res.results[c]["out"]).astype(np.float32)[None]
         for c in range(NCORES)], axis=0)
    return np.ascontiguousarray(out.reshape(B, O, K))
